# revision 38
# baseline (speedup 1.0000x reference)
"""MoE adapter layer (top-2 of 8 LoRA experts) for Trainium2, 8 NeuronCores.

Strategy
--------
Data-parallel over B: core b handles batch b (B == 8 == n_cores).

The reference's gating softmaxes masked logits where non-top-k entries are
-inf, so their gates are *exactly* 0.0 and only the top-2 experts per batch
contribute to the output.  Routing (an [8,1024]x[1024,8] matmul + top-2 +
softmax) is done on the host as part of input sharding; the two selected
rank-64 LoRAs of a batch are stacked into a single rank-128 LoRA, with the
gate weights folded into the up-projection:

    out[b].T = x[b].T + BwS_b @ (AS_b @ x[b].T)

where AS_b = concat(A[e0], A[e1]) is [128, H] and
BwS_b = concat(g0*Bw[e0], g1*Bw[e1]) is [H, 128].

On-device (per core), everything is done transposed (x.T is [H, L]) so the
contraction dim H lands on SBUF partitions for matmul 1 and the rank-128 mid
result lands on partitions for matmul 2.  The mm1 / mm2 / residual / store
pipeline is streamed over L blocks of 512 so output DMA overlaps input DMA
(the kernel is HBM-bandwidth bound: ~9 MB of bf16 traffic at ~358 GB/s/core).
x arrives in column groups (small first groups unlock the first L-block
early), with adjacent row-chunk pairs interleaved into [128, 2, w] tiles so
movers and stores handle two h-chunks per instruction.  A chain of tiny
warm-up matmuls at kernel start keeps the PE busy through the load phase so
HAM unthrottles it to 2.4 GHz before the real matmuls run.
"""

import os

import numpy as np

B, L, H = 8, 2048, 1024
E, TOPK, R = 8, 2, 64
P = 128
NF = 512  # matmul moving free dim / L block size
KH = H // P  # 8 contraction chunks over H
HC = H // P  # 8 output-row chunks over H
NLB = L // NF  # 4 L blocks
N_WARM = 64  # PE warm-up matmuls (N=128 each): long enough that HAM flips to
# 2.4 GHz *during* the warm-up chain (~3.4us of sustained PE busy) and the
# real matmul stream starts warm; mm1's DMA-arrival gaps otherwise keep
# resetting the HAM busy-window and the first half of the kernel runs at
# 1.2 GHz (427 ns/MM instead of 216 ns, measured)
# h-chunk pairs j whose residual goes through the PE as an identity matmul
# with a ScalarE copy as the mover (offloads VectorE at the cost of PE time);
# the rest add the residual on VectorE during the PSUM->SBUF move. (1, 3)
# measured best: () loses ~4us (VectorE serializes the mm2 tail), and
# 4-pair-on-ACT overloads the PE.
ACT_J = (1, 3)

# dtype config: "bf16" (bf16 I/O+matmuls, f32 PSUM accumulate),
# "f32r" (f32 I/O, float32r matmuls), "f32" (exact f32 matmuls, 4x slower PE)
CFG = os.environ.get("MOE_KERNEL_CFG", "bf16")

_BUILD_CACHE: dict = {}


def _dtypes(cfg):
    import concourse.mybir as mybir

    f32 = mybir.dt.float32
    if cfg == "bf16":
        bf16 = mybir.dt.bfloat16
        return dict(io=bf16, mm=bf16, mid=bf16, out=bf16, np_io=np.dtype("bfloat16"))
    if cfg == "f32r":
        f32r = mybir.dt.float32r
        return dict(io=f32r, mm=f32r, mid=f32r, out=f32, np_io=np.dtype(np.float32))
    if cfg == "f32":
        return dict(io=f32, mm=f32, mid=f32, out=f32, np_io=np.dtype(np.float32))
    raise ValueError(cfg)


def _build(cfg):
    """Build the single-core Bass program (same program SPMD on all 8 cores)."""
    if cfg in _BUILD_CACHE:
        return _BUILD_CACHE[cfg]

    import concourse.bacc as bacc
    import concourse.mybir as mybir
    from concourse.masks import make_identity
    from concourse.tile import TileContext

    dts = _dtypes(cfg)
    f32 = mybir.dt.float32

    # Bacc (not raw Bass): its compile() runs generate_event_semaphores,
    # which legalizes to TRN2's one-sync-wait-per-instruction limit.
    nc = bacc.Bacc()
    xT = nc.dram_tensor("xT", [H, L], dts["io"], kind="ExternalInput")
    # wA: AS.T pre-tiled on host as [p, k, m] = AS.T[k*128+p, m]
    wA = nc.dram_tensor("wA", [P, KH * P], dts["mm"], kind="ExternalInput")
    wB = nc.dram_tensor("wB", [P, H], dts["mm"], kind="ExternalInput")  # BwS.T
    yT = nc.dram_tensor("yT", [H, L], dts["out"], kind="ExternalOutput")

    def as_f32(ap):
        return ap.bitcast(f32) if ap.dtype == mybir.dt.float32r else ap

    with TileContext(nc) as tc:
        with (
            tc.tile_pool(name="wpool", bufs=1) as wpool,
            tc.tile_pool(name="xpool", bufs=KH // 2) as xpool,
            tc.tile_pool(name="midpool", bufs=3) as midpool,
            tc.tile_pool(name="outpool", bufs=HC) as outpool,
            tc.tile_pool(name="psA", bufs=2, space="PSUM") as psA,
            tc.tile_pool(name="psB", bufs=3, space="PSUM") as psB,
        ):
            # weights first, one per HWDGE ring
            wAt = wpool.tile([P, KH, P], dts["mm"], name="wAt")
            nc.sync.dma_start(out=wAt, in_=wA.rearrange("p (k m) -> p k m", k=KH))
            wBt = wpool.tile([P, H], dts["mm"], name="wB")
            nc.scalar.dma_start(out=wBt, in_=wB[:, :])

            # x loaded with adjacent row-chunk PAIRS interleaved into one tile
            # [128, 2, w]: partition r holds row r of chunk 2j (seg 0) and of
            # chunk 2j+1 (seg 1). This lets mm2's residual / mover / store
            # operate on h-chunk pairs as single [128, 2*NF] instructions.
            # Column groups: lb0, lb1 small (earliest pipeline unlock),
            # lb2-3 as one wide group. Loads alternate the two HWDGE rings.
            xgrp = {}
            for gi, (c0, c1) in enumerate(((0, NF), (NF, 2 * NF), (2 * NF, L))):
                for j in range(KH // 2):
                    t = xpool.tile(
                        [P, 2, c1 - c0], dts["io"], tag=f"xg{gi}", name=f"x{j}g{gi}"
                    )
                    eng = nc.sync if j % 2 == 0 else nc.scalar
                    eng.dma_start(
                        out=t,
                        in_=xT[2 * j * P : (2 * j + 2) * P, c0:c1].rearrange(
                            "(two p) c -> p two c", two=2
                        ),
                    )
                    xgrp[j, gi] = t

            def xpart(k, lb):
                """[128, NF] view of x row-chunk k, column block lb."""
                j, i = k // 2, k % 2
                if lb < 2:
                    return xgrp[j, lb][:, i, :]
                return xgrp[j, 2][:, i, (lb - 2) * NF : (lb - 1) * NF]

            def xpair(j, lb):
                """[128, 2, NF] view of x row-chunk pair j, column block lb."""
                if lb < 2:
                    return xgrp[j, lb][:, :, :]
                return xgrp[j, 2][:, :, (lb - 2) * NF : (lb - 1) * NF]

            # identity: warm-up operand + PE-side residual accumulate weights
            ident = wpool.tile([P, P], dts["mm"], name="ident")
            make_identity(nc, ident)

            # PE warm-up: a dependency-free chain of small matmuls that runs
            # while the x DMAs stream in, flipping HAM to 8/8 (2.4 GHz).
            # Output goes into a psA-pool slot (PSUM is fully subscribed).
            warm = wpool.tile([P, P], dts["mm"], name="warm")
            nc.vector.memset(warm, 1.0)
            warm_ps = psA.tile([P, NF], f32, tag="mid_ps", name="warm_ps")
            for _ in range(N_WARM):
                nc.tensor.matmul(
                    warm_ps[:, :P], lhsT=warm, rhs=warm, start=True, stop=True
                )

            for lb in range(NLB):
                ls = slice(lb * NF, (lb + 1) * NF)

                # mm1: mid[128, NF] = AS @ xT[:, ls], contract over H
                mid_ps = psA.tile([P, NF], f32, name="mid_ps")
                for k in range(KH):
                    nc.tensor.matmul(
                        mid_ps,
                        lhsT=wAt[:, k, :],
                        rhs=xpart(k, lb),
                        start=(k == 0),
                        stop=(k == KH - 1),
                    )
                mid_sb = midpool.tile([P, NF], dts["mid"], name="mid_sb")
                # alternate the mid mover so neither DVE nor ACT paces the
                # mm2 phase alone
                if lb % 2 == 0:
                    nc.scalar.copy(out=as_f32(mid_sb), in_=mid_ps)
                else:
                    nc.vector.tensor_copy(out=as_f32(mid_sb), in_=mid_ps)

                # mm2 + residual add + store, streamed per (h-pair, lb) so
                # output DMA overlaps the remaining input DMA; movers and
                # stores handle an h-chunk pair [128, 2*NF] per instruction
                for j in range(HC // 2):
                    out_ps = psB.tile([P, 2, NF], f32, name="out_ps")
                    on_act = j in ACT_J
                    for i in range(2):
                        h = 2 * j + i
                        nc.tensor.matmul(
                            out_ps[:, i, :],
                            lhsT=wBt[:, h * P : (h + 1) * P],
                            rhs=mid_sb,
                            start=True,
                            stop=not on_act,
                        )
                        if on_act:
                            # residual folded into PE; ScalarE moves the pair
                            nc.tensor.matmul(
                                out_ps[:, i, :],
                                lhsT=ident,
                                rhs=xpart(h, lb),
                                start=False,
                                stop=True,
                            )
                    out_sb = outpool.tile([P, 2, NF], dts["out"], name="out_sb")
                    if on_act:
                        nc.scalar.copy(out=out_sb, in_=out_ps)
                    else:
                        # residual added during the PSUM->SBUF move on VectorE
                        nc.vector.tensor_add(
                            out=out_sb, in0=out_ps, in1=as_f32(xpair(j, lb))
                        )
                    dma_eng = nc.gpsimd if j % 2 == 0 else nc.sync
                    dma_eng.dma_start(
                        out=yT[2 * j * P : (2 * j + 2) * P, ls].rearrange(
                            "(two p) c -> p two c", two=2
                        ),
                        in_=out_sb,
                    )

    nc.compile()
    _BUILD_CACHE[cfg] = nc
    return nc


def _route(x, Wr):
    """Host-side gating, mirroring the reference's noisy-top-k (eval) math."""
    cls = x[:, 0, :].astype(np.float32)  # [B, H]
    logits = cls @ Wr.T.astype(np.float32)  # [B, E]
    idx = np.argsort(-logits, axis=1, kind="stable")[:, :TOPK]  # [B, K] desc
    vals = np.take_along_axis(logits, idx, axis=1)
    e = np.exp(vals - vals.max(axis=1, keepdims=True))
    gates = e / e.sum(axis=1, keepdims=True)  # [B, K]
    return idx, gates.astype(np.float32)


def _ensure_ntff_hook_importable():
    """run_bass_kernel_spmd(trace=True) does a bare import of
    antenv.axon_hooks; some images lack it. Pre-install a shim (backed by the
    blessed ctypes NTFF hook when available) so tracing degrades gracefully
    instead of raising."""
    import sys

    try:
        from antenv.axon_hooks import get_axon_ntff_profile_hook  # noqa: F401

        return
    except ImportError:
        pass
    import types

    hook = None
    try:
        from trn_agent_boot.trn_boot import _ntff_profile_via_ctypes

        hook = _ntff_profile_via_ctypes("/opt/axon/libaxon_pjrt.so")
    except Exception:
        hook = None
    mod = types.ModuleType("antenv.axon_hooks")
    mod.get_axon_ntff_profile_hook = lambda: hook
    mod.set_axon_ntff_profile_hook = lambda h: None
    sys.modules["antenv.axon_hooks"] = mod


def kernel(x, Wr, A, Bw, _trace=False, _cfg=None):
    from concourse.bass_utils import run_bass_kernel_spmd

    _ensure_ntff_hook_importable()

    cfg = _cfg or CFG
    dts = _dtypes(cfg)
    np_io = dts["np_io"]

    x = np.asarray(x, dtype=np.float32)
    Wr = np.asarray(Wr, dtype=np.float32)
    A = np.asarray(A, dtype=np.float32)
    Bw = np.asarray(Bw, dtype=np.float32)

    idx, gates = _route(x, Wr)

    in_maps = []
    for b in range(B):
        e0, e1 = int(idx[b, 0]), int(idx[b, 1])
        g0, g1 = np.float32(gates[b, 0]), np.float32(gates[b, 1])
        AS = np.concatenate([A[e0], A[e1]], axis=0)  # [128, H]
        BwS = np.concatenate([g0 * Bw[e0], g1 * Bw[e1]], axis=1)  # [H, 128]
        # wA pre-tiled: [p, k*128+m] = AS.T[k*128+p, m] = AS[m, k*128+p]
        wAp = np.ascontiguousarray(
            AS.T.reshape(KH, P, P).transpose(1, 0, 2).reshape(P, KH * P)
        )
        in_maps.append(
            {
                "xT": np.ascontiguousarray(x[b].T).astype(np_io),
                "wA": wAp.astype(np_io),
                "wB": np.ascontiguousarray(BwS.T).astype(np_io),
            }
        )

    nc = _build(cfg)
    res = run_bass_kernel_spmd(
        nc,
        in_maps,
        core_ids=list(range(B)),
        trace=_trace,
        **({"trace_cores": list(range(B))} if _trace else {}),
    )

    out = np.empty((B, L, H), dtype=np.float32)
    for b in range(B):
        out[b] = res.results[b]["yT"].astype(np.float32).T
    if _trace:
        kernel._last_result = res
    return out


# revision 39
# speedup vs baseline: 1.0490x; 1.0490x over previous
"""MoE adapter layer (top-2 of 8 LoRA experts) for Trainium2, 8 NeuronCores.

Strategy
--------
Data-parallel over B: core b handles batch b (B == 8 == n_cores).

The reference's gating softmaxes masked logits where non-top-k entries are
-inf, so their gates are *exactly* 0.0 and only the top-2 experts per batch
contribute to the output.  Routing (an [8,1024]x[1024,8] matmul + top-2 +
softmax) is done on the host as part of input sharding; the two selected
rank-64 LoRAs of a batch are stacked into a single rank-128 LoRA, with the
gate weights folded into the up-projection:

    out[b].T = x[b].T + BwS_b @ (AS_b @ x[b].T)

where AS_b = concat(A[e0], A[e1]) is [128, H] and
BwS_b = concat(g0*Bw[e0], g1*Bw[e1]) is [H, 128].

On-device (per core), everything is done transposed (x.T is [H, L]) so the
contraction dim H lands on SBUF partitions for matmul 1 and the rank-128 mid
result lands on partitions for matmul 2.  The mm1 / mm2 / residual / store
pipeline is streamed over L blocks of 512 so output DMA overlaps input DMA
(the kernel is HBM-bandwidth bound: ~9 MB of bf16 traffic at ~358 GB/s/core).
x arrives in column groups (small first groups unlock the first L-block
early), with adjacent row-chunk pairs interleaved into [128, 2, w] tiles so
movers and stores handle two h-chunks per instruction.  A chain of tiny
warm-up matmuls at kernel start keeps the PE busy through the load phase so
HAM unthrottles it to 2.4 GHz before the real matmuls run.
"""

import os

import numpy as np

B, L, H = 8, 2048, 1024
E, TOPK, R = 8, 2, 64
P = 128
NF = 512  # matmul moving free dim / L block size
KH = H // P  # 8 contraction chunks over H
HC = H // P  # 8 output-row chunks over H
NLB = L // NF  # 4 L blocks
N_WARM = 64  # PE warm-up matmuls (N=128 each): long enough that HAM flips to
# 2.4 GHz *during* the warm-up chain (~3.4us of sustained PE busy) and the
# real matmul stream starts warm; mm1's DMA-arrival gaps otherwise keep
# resetting the HAM busy-window and the first half of the kernel runs at
# 1.2 GHz (427 ns/MM instead of 216 ns, measured)
# h-chunk pairs j whose residual goes through the PE as an identity matmul
# with a ScalarE copy as the mover (offloads VectorE at the cost of PE time);
# the rest add the residual on VectorE during the PSUM->SBUF move. (1, 3)
# measured best: () loses ~4us (VectorE serializes the mm2 tail), and
# 4-pair-on-ACT overloads the PE.
ACT_J = (1, 3)

# dtype config: "bf16" (bf16 I/O+matmuls, f32 PSUM accumulate),
# "f32r" (f32 I/O, float32r matmuls), "f32" (exact f32 matmuls, 4x slower PE)
CFG = os.environ.get("MOE_KERNEL_CFG", "bf16")

_BUILD_CACHE: dict = {}


def _dtypes(cfg):
    import concourse.mybir as mybir

    f32 = mybir.dt.float32
    if cfg == "bf16":
        bf16 = mybir.dt.bfloat16
        return dict(io=bf16, mm=bf16, mid=bf16, out=bf16, np_io=np.dtype("bfloat16"))
    if cfg == "f32r":
        f32r = mybir.dt.float32r
        return dict(io=f32r, mm=f32r, mid=f32r, out=f32, np_io=np.dtype(np.float32))
    if cfg == "f32":
        return dict(io=f32, mm=f32, mid=f32, out=f32, np_io=np.dtype(np.float32))
    raise ValueError(cfg)


def _build(cfg):
    """Build the single-core Bass program (same program SPMD on all 8 cores)."""
    if cfg in _BUILD_CACHE:
        return _BUILD_CACHE[cfg]

    import concourse.bacc as bacc
    import concourse.mybir as mybir
    from concourse.masks import make_identity
    from concourse.tile import TileContext

    dts = _dtypes(cfg)
    f32 = mybir.dt.float32

    # Bacc (not raw Bass): its compile() runs generate_event_semaphores,
    # which legalizes to TRN2's one-sync-wait-per-instruction limit.
    nc = bacc.Bacc()
    xT = nc.dram_tensor("xT", [H, L], dts["io"], kind="ExternalInput")
    # wA: AS.T pre-tiled on host as [p, k, m] = AS.T[k*128+p, m]
    wA = nc.dram_tensor("wA", [P, KH * P], dts["mm"], kind="ExternalInput")
    wB = nc.dram_tensor("wB", [P, H], dts["mm"], kind="ExternalInput")  # BwS.T
    yT = nc.dram_tensor("yT", [H, L], dts["out"], kind="ExternalOutput")

    def as_f32(ap):
        return ap.bitcast(f32) if ap.dtype == mybir.dt.float32r else ap

    with TileContext(nc) as tc:
        with (
            tc.tile_pool(name="wpool", bufs=1) as wpool,
            tc.tile_pool(name="xpool", bufs=KH // 2) as xpool,
            tc.tile_pool(name="midpool", bufs=3) as midpool,
            tc.tile_pool(name="outpool", bufs=HC) as outpool,
            tc.tile_pool(name="psA", bufs=2, space="PSUM") as psA,
            tc.tile_pool(name="psB", bufs=3, space="PSUM") as psB,
        ):
            # weights first, one per HWDGE ring
            wAt = wpool.tile([P, KH, P], dts["mm"], name="wAt")
            nc.sync.dma_start(out=wAt, in_=wA.rearrange("p (k m) -> p k m", k=KH))
            wBt = wpool.tile([P, H], dts["mm"], name="wB")
            nc.scalar.dma_start(out=wBt, in_=wB[:, :])

            # x loaded with adjacent row-chunk PAIRS interleaved into one tile
            # [128, 2, NF]: partition r holds row r of chunk 2j (seg 0) and of
            # chunk 2j+1 (seg 1). This lets mm2's residual / mover / store
            # operate on h-chunk pairs as single [128, 2*NF] instructions.
            # One column group per L-block, loaded L-block-major, so each
            # block's pipeline unlocks as soon as its own 4 tiles land (keeps
            # the PE stream dense enough that HAM stays at 2.4 GHz). Loads
            # alternate the two HWDGE rings.
            xgrp = {}
            for lb in range(NLB):
                for j in range(KH // 2):
                    t = xpool.tile(
                        [P, 2, NF], dts["io"], tag=f"xg{lb}", name=f"x{j}g{lb}"
                    )
                    eng = nc.sync if (lb * KH // 2 + j) % 2 == 0 else nc.scalar
                    eng.dma_start(
                        out=t,
                        in_=xT[
                            2 * j * P : (2 * j + 2) * P, lb * NF : (lb + 1) * NF
                        ].rearrange("(two p) c -> p two c", two=2),
                    )
                    xgrp[j, lb] = t

            def xpart(k, lb):
                """[128, NF] view of x row-chunk k, column block lb."""
                return xgrp[k // 2, lb][:, k % 2, :]

            def xpair(j, lb):
                """[128, 2, NF] view of x row-chunk pair j, column block lb."""
                return xgrp[j, lb][:, :, :]

            # identity: warm-up operand + PE-side residual accumulate weights
            ident = wpool.tile([P, P], dts["mm"], name="ident")
            make_identity(nc, ident)

            # PE warm-up: a dependency-free chain of small matmuls that runs
            # while the x DMAs stream in, flipping HAM to 8/8 (2.4 GHz).
            # Output goes into a psA-pool slot (PSUM is fully subscribed).
            warm = wpool.tile([P, P], dts["mm"], name="warm")
            nc.vector.memset(warm, 1.0)
            warm_ps = psA.tile([P, NF], f32, tag="mid_ps", name="warm_ps")
            for _ in range(N_WARM):
                nc.tensor.matmul(
                    warm_ps[:, :P], lhsT=warm, rhs=warm, start=True, stop=True
                )

            for lb in range(NLB):
                ls = slice(lb * NF, (lb + 1) * NF)

                # mm1: mid[128, NF] = AS @ xT[:, ls], contract over H
                mid_ps = psA.tile([P, NF], f32, name="mid_ps")
                for k in range(KH):
                    nc.tensor.matmul(
                        mid_ps,
                        lhsT=wAt[:, k, :],
                        rhs=xpart(k, lb),
                        start=(k == 0),
                        stop=(k == KH - 1),
                    )
                mid_sb = midpool.tile([P, NF], dts["mid"], name="mid_sb")
                # alternate the mid mover so neither DVE nor ACT paces the
                # mm2 phase alone
                if lb % 2 == 0:
                    nc.scalar.copy(out=as_f32(mid_sb), in_=mid_ps)
                else:
                    nc.vector.tensor_copy(out=as_f32(mid_sb), in_=mid_ps)

                # mm2 + residual add + store, streamed per (h-pair, lb) so
                # output DMA overlaps the remaining input DMA; movers and
                # stores handle an h-chunk pair [128, 2*NF] per instruction
                for j in range(HC // 2):
                    out_ps = psB.tile([P, 2, NF], f32, name="out_ps")
                    on_act = j in ACT_J
                    for i in range(2):
                        h = 2 * j + i
                        nc.tensor.matmul(
                            out_ps[:, i, :],
                            lhsT=wBt[:, h * P : (h + 1) * P],
                            rhs=mid_sb,
                            start=True,
                            stop=not on_act,
                        )
                        if on_act:
                            # residual folded into PE; ScalarE moves the pair
                            nc.tensor.matmul(
                                out_ps[:, i, :],
                                lhsT=ident,
                                rhs=xpart(h, lb),
                                start=False,
                                stop=True,
                            )
                    out_sb = outpool.tile([P, 2, NF], dts["out"], name="out_sb")
                    if on_act:
                        nc.scalar.copy(out=out_sb, in_=out_ps)
                    else:
                        # residual added during the PSUM->SBUF move on VectorE
                        nc.vector.tensor_add(
                            out=out_sb, in0=out_ps, in1=as_f32(xpair(j, lb))
                        )
                    dma_eng = nc.gpsimd if j % 2 == 0 else nc.sync
                    dma_eng.dma_start(
                        out=yT[2 * j * P : (2 * j + 2) * P, ls].rearrange(
                            "(two p) c -> p two c", two=2
                        ),
                        in_=out_sb,
                    )

    nc.compile()
    _BUILD_CACHE[cfg] = nc
    return nc


def _route(x, Wr):
    """Host-side gating, mirroring the reference's noisy-top-k (eval) math."""
    cls = x[:, 0, :].astype(np.float32)  # [B, H]
    logits = cls @ Wr.T.astype(np.float32)  # [B, E]
    idx = np.argsort(-logits, axis=1, kind="stable")[:, :TOPK]  # [B, K] desc
    vals = np.take_along_axis(logits, idx, axis=1)
    e = np.exp(vals - vals.max(axis=1, keepdims=True))
    gates = e / e.sum(axis=1, keepdims=True)  # [B, K]
    return idx, gates.astype(np.float32)


def _ensure_ntff_hook_importable():
    """run_bass_kernel_spmd(trace=True) does a bare import of
    antenv.axon_hooks; some images lack it. Pre-install a shim (backed by the
    blessed ctypes NTFF hook when available) so tracing degrades gracefully
    instead of raising."""
    import sys

    try:
        from antenv.axon_hooks import get_axon_ntff_profile_hook  # noqa: F401

        return
    except ImportError:
        pass
    import types

    hook = None
    try:
        from trn_agent_boot.trn_boot import _ntff_profile_via_ctypes

        hook = _ntff_profile_via_ctypes("/opt/axon/libaxon_pjrt.so")
    except Exception:
        hook = None
    mod = types.ModuleType("antenv.axon_hooks")
    mod.get_axon_ntff_profile_hook = lambda: hook
    mod.set_axon_ntff_profile_hook = lambda h: None
    sys.modules["antenv.axon_hooks"] = mod


def kernel(x, Wr, A, Bw, _trace=False, _cfg=None):
    from concourse.bass_utils import run_bass_kernel_spmd

    _ensure_ntff_hook_importable()

    cfg = _cfg or CFG
    dts = _dtypes(cfg)
    np_io = dts["np_io"]

    x = np.asarray(x, dtype=np.float32)
    Wr = np.asarray(Wr, dtype=np.float32)
    A = np.asarray(A, dtype=np.float32)
    Bw = np.asarray(Bw, dtype=np.float32)

    idx, gates = _route(x, Wr)

    in_maps = []
    for b in range(B):
        e0, e1 = int(idx[b, 0]), int(idx[b, 1])
        g0, g1 = np.float32(gates[b, 0]), np.float32(gates[b, 1])
        AS = np.concatenate([A[e0], A[e1]], axis=0)  # [128, H]
        BwS = np.concatenate([g0 * Bw[e0], g1 * Bw[e1]], axis=1)  # [H, 128]
        # wA pre-tiled: [p, k*128+m] = AS.T[k*128+p, m] = AS[m, k*128+p]
        wAp = np.ascontiguousarray(
            AS.T.reshape(KH, P, P).transpose(1, 0, 2).reshape(P, KH * P)
        )
        in_maps.append(
            {
                "xT": np.ascontiguousarray(x[b].T).astype(np_io),
                "wA": wAp.astype(np_io),
                "wB": np.ascontiguousarray(BwS.T).astype(np_io),
            }
        )

    nc = _build(cfg)
    res = run_bass_kernel_spmd(
        nc,
        in_maps,
        core_ids=list(range(B)),
        trace=_trace,
        **({"trace_cores": list(range(B))} if _trace else {}),
    )

    out = np.empty((B, L, H), dtype=np.float32)
    for b in range(B):
        out[b] = res.results[b]["yT"].astype(np.float32).T
    if _trace:
        kernel._last_result = res
    return out


# revision 42
# speedup vs baseline: 1.0550x; 1.0057x over previous
"""MoE adapter layer (top-2 of 8 LoRA experts) for Trainium2, 8 NeuronCores.

Strategy
--------
Data-parallel over B: core b handles batch b (B == 8 == n_cores).

The reference's gating softmaxes masked logits where non-top-k entries are
-inf, so their gates are *exactly* 0.0 and only the top-2 experts per batch
contribute to the output.  Routing (an [8,1024]x[1024,8] matmul + top-2 +
softmax) is done on the host as part of input sharding; the two selected
rank-64 LoRAs of a batch are stacked into a single rank-128 LoRA, with the
gate weights folded into the up-projection:

    out[b].T = x[b].T + BwS_b @ (AS_b @ x[b].T)

where AS_b = concat(A[e0], A[e1]) is [128, H] and
BwS_b = concat(g0*Bw[e0], g1*Bw[e1]) is [H, 128].

On-device (per core), everything is done transposed (x.T is [H, L]) so the
contraction dim H lands on SBUF partitions for matmul 1 and the rank-128 mid
result lands on partitions for matmul 2.  The mm1 / mm2 / residual / store
pipeline is streamed over L blocks of 512 so output DMA overlaps input DMA
(the kernel is HBM-bandwidth bound: ~9 MB of bf16 traffic at ~358 GB/s/core).
x arrives in column groups (small first groups unlock the first L-block
early), with adjacent row-chunk pairs interleaved into [128, 2, w] tiles so
movers and stores handle two h-chunks per instruction.  A chain of tiny
warm-up matmuls at kernel start keeps the PE busy through the load phase so
HAM unthrottles it to 2.4 GHz before the real matmuls run.
"""

import os

import numpy as np

B, L, H = 8, 2048, 1024
E, TOPK, R = 8, 2, 64
P = 128
NF = 512  # matmul moving free dim / L block size
KH = H // P  # 8 contraction chunks over H
HC = H // P  # 8 output-row chunks over H
NLB = L // NF  # 4 L blocks
N_WARM = 48  # PE warm-up matmuls (N=128 each): long enough that HAM flips to
# 2.4 GHz *during* the warm-up chain (~3.4us of sustained PE busy) and the
# real matmul stream starts warm; mm1's DMA-arrival gaps otherwise keep
# resetting the HAM busy-window and the first half of the kernel runs at
# 1.2 GHz (427 ns/MM instead of 216 ns, measured)
# h-chunk pairs j whose residual goes through the PE as an identity matmul
# with a ScalarE copy as the mover (offloads VectorE at the cost of PE time);
# the rest add the residual on VectorE during the PSUM->SBUF move. (1, 3)
# measured best: () loses ~4us (VectorE serializes the mm2 tail), and
# 4-pair-on-ACT overloads the PE.
ACT_J = (1, 3)

# dtype config: "bf16" (bf16 I/O+matmuls, f32 PSUM accumulate),
# "f32r" (f32 I/O, float32r matmuls), "f32" (exact f32 matmuls, 4x slower PE)
CFG = os.environ.get("MOE_KERNEL_CFG", "bf16")

_BUILD_CACHE: dict = {}


def _dtypes(cfg):
    import concourse.mybir as mybir

    f32 = mybir.dt.float32
    if cfg == "bf16":
        bf16 = mybir.dt.bfloat16
        return dict(io=bf16, mm=bf16, mid=bf16, out=bf16, np_io=np.dtype("bfloat16"))
    if cfg == "f32r":
        f32r = mybir.dt.float32r
        return dict(io=f32r, mm=f32r, mid=f32r, out=f32, np_io=np.dtype(np.float32))
    if cfg == "f32":
        return dict(io=f32, mm=f32, mid=f32, out=f32, np_io=np.dtype(np.float32))
    raise ValueError(cfg)


def _build(cfg):
    """Build the single-core Bass program (same program SPMD on all 8 cores)."""
    if cfg in _BUILD_CACHE:
        return _BUILD_CACHE[cfg]

    import concourse.bacc as bacc
    import concourse.mybir as mybir
    from concourse.masks import make_identity
    from concourse.tile import TileContext

    dts = _dtypes(cfg)
    f32 = mybir.dt.float32

    # Bacc (not raw Bass): its compile() runs generate_event_semaphores,
    # which legalizes to TRN2's one-sync-wait-per-instruction limit.
    nc = bacc.Bacc()
    xT = nc.dram_tensor("xT", [H, L], dts["io"], kind="ExternalInput")
    # wA: AS.T pre-tiled on host as [p, k, m] = AS.T[k*128+p, m]
    wA = nc.dram_tensor("wA", [P, KH * P], dts["mm"], kind="ExternalInput")
    wB = nc.dram_tensor("wB", [P, H], dts["mm"], kind="ExternalInput")  # BwS.T
    yT = nc.dram_tensor("yT", [H, L], dts["out"], kind="ExternalOutput")

    def as_f32(ap):
        return ap.bitcast(f32) if ap.dtype == mybir.dt.float32r else ap

    with TileContext(nc) as tc:
        with (
            tc.tile_pool(name="wpool", bufs=1) as wpool,
            tc.tile_pool(name="xpool", bufs=KH // 2) as xpool,
            tc.tile_pool(name="midpool", bufs=3) as midpool,
            tc.tile_pool(name="outpool", bufs=HC) as outpool,
            tc.tile_pool(name="psA", bufs=2, space="PSUM") as psA,
            tc.tile_pool(name="psB", bufs=3, space="PSUM") as psB,
        ):
            # wA first (mm1 k=0 needs it); wB is deferred until after the
            # first two L-blocks' x tiles — it isn't needed until mm2 and
            # loading it up front delays lb0's arrival on the scalar ring
            wAt = wpool.tile([P, KH, P], dts["mm"], name="wAt")
            nc.sync.dma_start(out=wAt, in_=wA.rearrange("p (k m) -> p k m", k=KH))
            wBt = wpool.tile([P, H], dts["mm"], name="wB")

            # x loaded with adjacent row-chunk PAIRS interleaved into one tile
            # [128, 2, NF]: partition r holds row r of chunk 2j (seg 0) and of
            # chunk 2j+1 (seg 1). This lets mm2's residual / mover / store
            # operate on h-chunk pairs as single [128, 2*NF] instructions.
            # One column group per L-block, loaded L-block-major, so each
            # block's pipeline unlocks as soon as its own 4 tiles land (keeps
            # the PE stream dense enough that HAM stays at 2.4 GHz). Loads
            # alternate the two HWDGE rings.
            xgrp = {}
            for lb in range(NLB):
                for j in range(KH // 2):
                    t = xpool.tile(
                        [P, 2, NF], dts["io"], tag=f"xg{lb}", name=f"x{j}g{lb}"
                    )
                    eng = nc.sync if (lb * KH // 2 + j) % 2 == 0 else nc.scalar
                    eng.dma_start(
                        out=t,
                        in_=xT[
                            2 * j * P : (2 * j + 2) * P, lb * NF : (lb + 1) * NF
                        ].rearrange("(two p) c -> p two c", two=2),
                    )
                    xgrp[j, lb] = t
                if lb == 1:
                    nc.scalar.dma_start(out=wBt, in_=wB[:, :])

            def xpart(k, lb):
                """[128, NF] view of x row-chunk k, column block lb."""
                return xgrp[k // 2, lb][:, k % 2, :]

            def xpair(j, lb):
                """[128, 2, NF] view of x row-chunk pair j, column block lb."""
                return xgrp[j, lb][:, :, :]

            # identity: warm-up operand + PE-side residual accumulate weights
            ident = wpool.tile([P, P], dts["mm"], name="ident")
            make_identity(nc, ident)

            # PE warm-up: a dependency-free chain of small matmuls that runs
            # while the x DMAs stream in, flipping HAM to 8/8 (2.4 GHz).
            # Output goes into a psA-pool slot (PSUM is fully subscribed).
            warm = wpool.tile([P, P], dts["mm"], name="warm")
            nc.vector.memset(warm, 1.0)
            warm_ps = psA.tile([P, NF], f32, tag="mid_ps", name="warm_ps")
            for _ in range(N_WARM):
                nc.tensor.matmul(
                    warm_ps[:, :P], lhsT=warm, rhs=warm, start=True, stop=True
                )

            for lb in range(NLB):
                ls = slice(lb * NF, (lb + 1) * NF)

                # mm1: mid[128, NF] = AS @ xT[:, ls], contract over H
                mid_ps = psA.tile([P, NF], f32, name="mid_ps")
                for k in range(KH):
                    nc.tensor.matmul(
                        mid_ps,
                        lhsT=wAt[:, k, :],
                        rhs=xpart(k, lb),
                        start=(k == 0),
                        stop=(k == KH - 1),
                    )
                mid_sb = midpool.tile([P, NF], dts["mid"], name="mid_sb")
                # alternate the mid mover so neither DVE nor ACT paces the
                # mm2 phase alone
                if lb % 2 == 0:
                    nc.scalar.copy(out=as_f32(mid_sb), in_=mid_ps)
                else:
                    nc.vector.tensor_copy(out=as_f32(mid_sb), in_=mid_ps)

                # mm2 + residual add + store, streamed per (h-pair, lb) so
                # output DMA overlaps the remaining input DMA; movers and
                # stores handle an h-chunk pair [128, 2*NF] per instruction
                for j in range(HC // 2):
                    out_ps = psB.tile([P, 2, NF], f32, name="out_ps")
                    on_act = j in ACT_J
                    for i in range(2):
                        h = 2 * j + i
                        nc.tensor.matmul(
                            out_ps[:, i, :],
                            lhsT=wBt[:, h * P : (h + 1) * P],
                            rhs=mid_sb,
                            start=True,
                            stop=not on_act,
                        )
                        if on_act:
                            # residual folded into PE; ScalarE moves the pair
                            nc.tensor.matmul(
                                out_ps[:, i, :],
                                lhsT=ident,
                                rhs=xpart(h, lb),
                                start=False,
                                stop=True,
                            )
                    out_sb = outpool.tile([P, 2, NF], dts["out"], name="out_sb")
                    if on_act:
                        nc.scalar.copy(out=out_sb, in_=out_ps)
                    else:
                        # residual added during the PSUM->SBUF move on VectorE
                        nc.vector.tensor_add(
                            out=out_sb, in0=out_ps, in1=as_f32(xpair(j, lb))
                        )
                    dma_eng = nc.gpsimd if j % 2 == 0 else nc.sync
                    dma_eng.dma_start(
                        out=yT[2 * j * P : (2 * j + 2) * P, ls].rearrange(
                            "(two p) c -> p two c", two=2
                        ),
                        in_=out_sb,
                    )

    nc.compile()
    _BUILD_CACHE[cfg] = nc
    return nc


def _route(x, Wr):
    """Host-side gating, mirroring the reference's noisy-top-k (eval) math."""
    cls = x[:, 0, :].astype(np.float32)  # [B, H]
    logits = cls @ Wr.T.astype(np.float32)  # [B, E]
    idx = np.argsort(-logits, axis=1, kind="stable")[:, :TOPK]  # [B, K] desc
    vals = np.take_along_axis(logits, idx, axis=1)
    e = np.exp(vals - vals.max(axis=1, keepdims=True))
    gates = e / e.sum(axis=1, keepdims=True)  # [B, K]
    return idx, gates.astype(np.float32)


def _ensure_ntff_hook_importable():
    """run_bass_kernel_spmd(trace=True) does a bare import of
    antenv.axon_hooks; some images lack it. Pre-install a shim (backed by the
    blessed ctypes NTFF hook when available) so tracing degrades gracefully
    instead of raising."""
    import sys

    try:
        from antenv.axon_hooks import get_axon_ntff_profile_hook  # noqa: F401

        return
    except ImportError:
        pass
    import types

    hook = None
    try:
        from trn_agent_boot.trn_boot import _ntff_profile_via_ctypes

        hook = _ntff_profile_via_ctypes("/opt/axon/libaxon_pjrt.so")
    except Exception:
        hook = None
    mod = types.ModuleType("antenv.axon_hooks")
    mod.get_axon_ntff_profile_hook = lambda: hook
    mod.set_axon_ntff_profile_hook = lambda h: None
    sys.modules["antenv.axon_hooks"] = mod


def kernel(x, Wr, A, Bw, _trace=False, _cfg=None):
    from concourse.bass_utils import run_bass_kernel_spmd

    _ensure_ntff_hook_importable()

    cfg = _cfg or CFG
    dts = _dtypes(cfg)
    np_io = dts["np_io"]

    x = np.asarray(x, dtype=np.float32)
    Wr = np.asarray(Wr, dtype=np.float32)
    A = np.asarray(A, dtype=np.float32)
    Bw = np.asarray(Bw, dtype=np.float32)

    idx, gates = _route(x, Wr)

    in_maps = []
    for b in range(B):
        e0, e1 = int(idx[b, 0]), int(idx[b, 1])
        g0, g1 = np.float32(gates[b, 0]), np.float32(gates[b, 1])
        AS = np.concatenate([A[e0], A[e1]], axis=0)  # [128, H]
        BwS = np.concatenate([g0 * Bw[e0], g1 * Bw[e1]], axis=1)  # [H, 128]
        # wA pre-tiled: [p, k*128+m] = AS.T[k*128+p, m] = AS[m, k*128+p]
        wAp = np.ascontiguousarray(
            AS.T.reshape(KH, P, P).transpose(1, 0, 2).reshape(P, KH * P)
        )
        in_maps.append(
            {
                "xT": np.ascontiguousarray(x[b].T).astype(np_io),
                "wA": wAp.astype(np_io),
                "wB": np.ascontiguousarray(BwS.T).astype(np_io),
            }
        )

    nc = _build(cfg)
    res = run_bass_kernel_spmd(
        nc,
        in_maps,
        core_ids=list(range(B)),
        trace=_trace,
        **({"trace_cores": list(range(B))} if _trace else {}),
    )

    out = np.empty((B, L, H), dtype=np.float32)
    for b in range(B):
        out[b] = res.results[b]["yT"].astype(np.float32).T
    if _trace:
        kernel._last_result = res
    return out
